# revision 13
# baseline (speedup 1.0000x reference)
"""Trainium2 Bass kernel for the Dale's-law CustomRNN.

Math (per reference):
    drive = x @ W_in.T + bias + noise_seq                    [B, T, N]
    r_{t+1} = (1-a) r_t + a relu(r_t @ W_rec.T + drive_t)    a = 0.2
    z_t = r_{t+1} @ W_out_w.T + W_out_b                      [B, T, 3]
    l2 = mean_t mean_{B,N} (r_{t+1}^2)

Device strategy (8 cores, data-parallel over batch, 128 batch/core):
  - State in [hidden-on-partitions, batch-on-free] layout, two hidden
    halves; no transposes anywhere.
  - One PSUM bank per (step, half) holds a*(pre + drive): drive
    (= a*(x@W_in.T + noise + bias), prepared host-side during
    sharding/layout) injected via identity matmul, recurrent matmuls
    accumulate on top.
  - Chain-shortening decomposition: a*W r_t = [b*a*W] r_{t-1} +
    [a*W] s_{t-1} (b = 0.8): serial loop is relu -> s-matmuls -> relu
    (2 sem hops); the state fma runs off the critical path on VectorE.
    Halves are staggered: relu(h0) overlaps the h1 matmuls.
  - z = W_out @ r batched per 4 steps on PE, copied out by ScalarE;
    l2 via square+accum_out on VectorE in small pieces (avoids FIFO
    head-of-line blocking of the fma).
  - Matmul operands fp16 (PE 1 cycle/row + FWL); PSUM fp32; state fp16
    (validated ~1e-3 rel err vs fp32 reference).
"""

import sys

sys.path.insert(0, "/opt/trn_rl_repo")

import numpy as np

ALPHA = 10.0 / 50.0
BETA = 1.0 - ALPHA
B_FULL = 1024
T_FULL = 512
N_REC = 256
N_IN = 10
N_OUT = 3
NCORES = 8
BC = B_FULL // NCORES  # 128
TCHUNK = 16

_NC_CACHE = {}


def _build(T):
    import concourse.tile as tile
    from concourse import mybir, bacc

    f16 = mybir.dt.float16
    f32 = mybir.dt.float32
    mult = mybir.AluOpType.mult
    add = mybir.AluOpType.add
    Relu = mybir.ActivationFunctionType.Relu

    nch = T // TCHUNK
    nc = bacc.Bacc("TRN2", target_bir_lowering=False, debug=False, num_devices=NCORES)

    drive_d = nc.dram_tensor("drive", [128, T, 2, 128], f16, kind="ExternalInput")
    # wrec holds two scaled copies of W_rec.T: [0] = a*W, [1] = b*a*W
    wrec_d = nc.dram_tensor("wrec", [128, 2, 2, 2, 128], f16, kind="ExternalInput")
    id_d = nc.dram_tensor("ident", [128, 128], f16, kind="ExternalInput")
    wout_d = nc.dram_tensor("wout", [128, 2, 3], f16, kind="ExternalInput")
    z_d = nc.dram_tensor("z", [3, T, 128], f32, kind="ExternalOutput")
    l2_d = nc.dram_tensor("l2", [128, T // 2], f32, kind="ExternalOutput")

    with tile.TileContext(nc) as tc:
        with (
            tc.tile_pool(name="const", bufs=1) as const_p,
            tc.tile_pool(name="drive", bufs=3) as drive_p,
            tc.tile_pool(name="rhist", bufs=2) as r_p,
            tc.tile_pool(name="s", bufs=4) as s_p,
            tc.tile_pool(name="zs", bufs=2) as z_p,
            tc.tile_pool(name="sq", bufs=2) as sq_p,
            tc.tile_pool(name="acc", bufs=1) as acc_p,
            tc.tile_pool(name="ups", bufs=6, space="PSUM") as ups_p,
            tc.tile_pool(name="zps", bufs=2, space="PSUM") as zps_p,
        ):
            wrec_t = const_p.tile([128, 2, 2, 2, 128], f16)
            nc.sync.dma_start(out=wrec_t[:], in_=wrec_d[:])
            id_t = const_p.tile([128, 128], f16)
            nc.sync.dma_start(out=id_t[:], in_=id_d[:])
            wout_t = const_p.tile([128, 2, 3], f16)
            nc.sync.dma_start(out=wout_t[:], in_=wout_d[:])
            l2acc = acc_p.tile([128, T // 2], f32)

            rv = None       # r_t (state entering current step), [128,2,128] AP
            rv_prev = None  # r_{t-1}
            sv = None       # s_{t-1}, tuple of half APs
            for ch in range(nch):
                t0 = ch * TCHUNK
                drive_t = drive_p.tile([128, TCHUNK, 2, 128], f16)
                nc.sync.dma_start(out=drive_t[:], in_=drive_d[:, t0 : t0 + TCHUNK, :, :])
                rh = r_p.tile([128, TCHUNK, 2, 128], f16)
                z_sb = z_p.tile([3, TCHUNK, 128], f32)
                ups_pair = {}
                for lt in range(TCHUNK):
                    t = t0 + lt
                    s_t = s_p.tile([128, 2, 128], f16, tag="s")
                    # one PSUM bank per step holds both hidden halves;
                    # pairs of banks are drive-injected together so the
                    # identity LDWEIGHTS is shared between two injects
                    if lt % 2 == 0:
                        for j in (0, 1):
                            upj = ups_p.tile([128, 2, 128], f32, tag="ups")
                            ups_pair[lt + j] = upj
                            nc.tensor.matmul(
                                upj[:], id_t[:], drive_t[:, lt + j, :, :],
                                start=True, stop=(sv is None and j == 0 and lt == 0),
                            )
                    up = ups_pair.pop(lt)
                    # r-matmuls: operand ready ~2 steps early
                    if rv_prev is not None:
                        for h in range(2):
                            for kc in range(2):
                                nc.tensor.matmul(
                                    up[:, h, :], wrec_t[:, 1, kc, h, :],
                                    rv_prev[:, kc, :],
                                    start=False, stop=False,
                                )
                    # s-matmuls: the chain edge (need s_{t-1})
                    if sv is not None:
                        for h in range(2):
                            for kc in range(2):
                                nc.tensor.matmul(
                                    up[:, h, :], wrec_t[:, 0, kc, h, :],
                                    sv[kc],
                                    start=False, stop=(h == 1 and kc == 1),
                                )
                    nc.scalar.activation(s_t[:], up[:], Relu)
                    # state update (off critical path): rh[t] = r_{t+1}
                    if rv is None:
                        nc.vector.tensor_copy(rh[:, lt, :, :], s_t[:])  # r_1 = s_0
                    else:
                        nc.vector.scalar_tensor_tensor(
                            out=rh[:, lt, :, :], in0=rv, scalar=BETA,
                            in1=s_t[:], op0=mult, op1=add,
                        )
                    rv_prev = rv  # None at t<=1 skips the r-MMs (r_0 = 0)
                    rv = rh[:, lt, :, :]
                    sv = (s_t[:, 0, :], s_t[:, 1, :])
                    # l2 partial sums per 2 steps (small pieces: avoid
                    # blocking the chain in either FIFO); alternate between
                    # VectorE (square via STT) and ScalarE (Square func),
                    # both with accum_out, to balance engine load
                    if lt % 2 == 1:
                        sq_t = sq_p.tile([128, 2, 2, 128], f16, tag="sq")
                        sl = rh[:, lt - 1 : lt + 1, :, :]
                        if (lt // 2) % 2 == 0:
                            nc.vector.scalar_tensor_tensor(
                                out=sq_t[:], in0=sl, scalar=1.0, in1=sl,
                                op0=mult, op1=mult,
                                accum_out=l2acc[:, t // 2 : t // 2 + 1],
                            )
                        else:
                            nc.scalar.activation(
                                sq_t[:], sl,
                                mybir.ActivationFunctionType.Square,
                                accum_out=l2acc[:, t // 2 : t // 2 + 1],
                            )
                    # z per completed 4-step window
                    if lt % 4 == 3:
                        q = lt // 4
                        zp = zps_p.tile([3, 4, 128], f32, tag="zp")
                        for kc in range(2):
                            nc.tensor.matmul(
                                zp[:],
                                wout_t[:, kc, :],
                                rh[:, 4 * q : 4 * q + 4, kc, :],
                                start=(kc == 0),
                                stop=(kc == 1),
                            )
                        nc.scalar.copy(z_sb[:, 4 * q : 4 * q + 4, :], zp[:])
                nc.sync.dma_start(out=z_d[:, t0 : t0 + TCHUNK, :], in_=z_sb[:])
            nc.sync.dma_start(out=l2_d[:], in_=l2acc[:])
    nc.compile()
    return nc


def _get_nc(T):
    if T not in _NC_CACHE:
        _NC_CACHE[T] = _build(T)
    return _NC_CACHE[T]


def _prep_maps(x, noise_seq, W_in, W_rec, W_out_w, bias):
    T = x.shape[1]
    a = np.float32(ALPHA)
    wrec = np.empty([128, 2, 2, 2, 128], np.float16)
    for kc in range(2):
        for h in range(2):
            blk = W_rec[h * 128 : (h + 1) * 128, kc * 128 : (kc + 1) * 128]
            wrec[:, 0, kc, h, :] = (a * blk.T).astype(np.float16)
            wrec[:, 1, kc, h, :] = (np.float32(BETA) * a * blk.T).astype(np.float16)
    ident = np.eye(128, dtype=np.float16)
    wout = np.empty([128, 2, 3], np.float16)
    wout[:, 0, :] = W_out_w[:, 0:128].T.astype(np.float16)
    wout[:, 1, :] = W_out_w[:, 128:256].T.astype(np.float16)

    in_maps = []
    for c in range(NCORES):
        bs = slice(c * BC, (c + 1) * BC)
        xw = np.matmul(x[bs], W_in.T.astype(np.float32))  # [128, T, 256]
        dr = a * (noise_seq[bs] + bias[None, None, :] + xw)
        drt = dr.transpose(2, 1, 0).reshape(2, 128, T, 128)  # [h, p, t, b]
        drive = np.ascontiguousarray(drt.transpose(1, 2, 0, 3), dtype=np.float16)
        in_maps.append(
            {"drive": drive, "wrec": wrec, "ident": ident, "wout": wout}
        )
    return in_maps


def install_profile_hook():
    """Register the axon NTFF profiling hook (the agent image lacks the
    antenv.axon_hooks module concourse expects; the C ABI is present in
    the injected libaxon_pjrt.so)."""
    import types

    if "antenv.axon_hooks" in sys.modules:
        return
    sys.path.insert(0, "/root/.axon_site")
    from trn_agent_boot import trn_boot

    hook = trn_boot._ntff_profile_via_ctypes("/opt/axon/libaxon_pjrt.so")
    mod = types.ModuleType("antenv.axon_hooks")
    mod.get_axon_ntff_profile_hook = lambda: hook
    mod.set_axon_ntff_profile_hook = lambda h: None
    sys.modules["antenv.axon_hooks"] = mod


def run(inputs, trace=False):
    """Returns ((out, l2_rates), BassKernelResults)."""
    from concourse.bass_utils import run_bass_kernel_spmd

    if trace:
        install_profile_hook()

    x = np.asarray(inputs["x"], np.float32)
    noise_seq = np.asarray(inputs["noise_seq"], np.float32)
    W_in = np.asarray(inputs["W_in"], np.float32)
    W_rec = np.asarray(inputs["W_rec"], np.float32)
    W_out_w = np.asarray(inputs["W_out_w"], np.float32)
    W_out_b = np.asarray(inputs["W_out_b"], np.float32)
    bias = np.asarray(inputs["bias"], np.float32)
    T = x.shape[1]
    Bt = x.shape[0]
    assert Bt == B_FULL, Bt

    nc = _get_nc(T)
    in_maps = _prep_maps(x, noise_seq, W_in, W_rec, W_out_w, bias)
    res = run_bass_kernel_spmd(
        nc, in_maps, core_ids=list(range(NCORES)), trace=trace
    )

    out = np.empty([B_FULL, T, N_OUT], np.float32)
    tot = 0.0
    for c in range(NCORES):
        z = res.results[c]["z"]  # [3, T, 128]
        out[c * BC : (c + 1) * BC] = z.transpose(2, 1, 0)
        tot += res.results[c]["l2"].sum(dtype=np.float64)
    out += W_out_b[None, None, :]
    l2 = np.float32(tot / (T * B_FULL * N_REC))
    return (out, l2), res


def kernel(**inputs):
    (out, l2), _ = run(inputs, trace=False)
    return out, l2


# revision 14
# speedup vs baseline: 1.1289x; 1.1289x over previous
"""Trainium2 Bass kernel for the Dale's-law CustomRNN.

Math (per reference):
    drive = x @ W_in.T + bias + noise_seq                    [B, T, N]
    r_{t+1} = (1-a) r_t + a relu(r_t @ W_rec.T + drive_t)    a = 0.2
    z_t = r_{t+1} @ W_out_w.T + W_out_b                      [B, T, 3]
    l2 = mean_t mean_{B,N} (r_{t+1}^2)

Device strategy (8 cores, data-parallel over batch, 128 batch/core):
  - State in [hidden-on-partitions, batch-on-free] layout, two hidden
    halves; no transposes anywhere.
  - One PSUM bank per (step, half) holds a*(pre + drive): drive
    (= a*(x@W_in.T + noise + bias), prepared host-side during
    sharding/layout) injected via identity matmul, recurrent matmuls
    accumulate on top.
  - Chain-shortening decomposition: a*W r_t = [b*a*W] r_{t-1} +
    [a*W] s_{t-1} (b = 0.8): serial loop is relu -> s-matmuls -> relu
    (2 sem hops); the state fma runs off the critical path on VectorE.
    Halves are staggered: relu(h0) overlaps the h1 matmuls.
  - z = W_out @ r batched per 4 steps on PE, copied out by ScalarE;
    l2 via square+accum_out on VectorE in small pieces (avoids FIFO
    head-of-line blocking of the fma).
  - Matmul operands fp16 (PE 1 cycle/row + FWL); PSUM fp32; state fp16
    (validated ~1e-3 rel err vs fp32 reference).
"""

import sys

sys.path.insert(0, "/opt/trn_rl_repo")

import numpy as np

ALPHA = 10.0 / 50.0
BETA = 1.0 - ALPHA
B_FULL = 1024
T_FULL = 512
N_REC = 256
N_IN = 10
N_OUT = 3
NCORES = 8
BC = B_FULL // NCORES  # 128
TCHUNK = 16

_NC_CACHE = {}


def _build(T):
    import concourse.tile as tile
    from concourse import mybir, bacc

    f16 = mybir.dt.float16
    f32 = mybir.dt.float32
    mult = mybir.AluOpType.mult
    add = mybir.AluOpType.add
    Relu = mybir.ActivationFunctionType.Relu

    nch = T // TCHUNK
    nc = bacc.Bacc("TRN2", target_bir_lowering=False, debug=False, num_devices=NCORES)

    drive_d = nc.dram_tensor("drive", [128, T, 2, 128], f16, kind="ExternalInput")
    # wrec holds two scaled copies of W_rec.T: [0] = a*W, [1] = b*a*W
    wrec_d = nc.dram_tensor("wrec", [128, 2, 2, 2, 128], f16, kind="ExternalInput")
    id_d = nc.dram_tensor("ident", [128, 128], f16, kind="ExternalInput")
    wout_d = nc.dram_tensor("wout", [128, 2, 3], f16, kind="ExternalInput")
    z_d = nc.dram_tensor("z", [3, T, 128], f32, kind="ExternalOutput")
    l2_d = nc.dram_tensor("l2", [128, T // 2], f32, kind="ExternalOutput")

    with tile.TileContext(nc) as tc:
        with (
            tc.tile_pool(name="const", bufs=1) as const_p,
            tc.tile_pool(name="drive", bufs=3) as drive_p,
            tc.tile_pool(name="rhist", bufs=2) as r_p,
            tc.tile_pool(name="s", bufs=4) as s_p,
            tc.tile_pool(name="zs", bufs=2) as z_p,
            tc.tile_pool(name="sq", bufs=2) as sq_p,
            tc.tile_pool(name="acc", bufs=1) as acc_p,
            tc.tile_pool(name="ups", bufs=6, space="PSUM") as ups_p,
            tc.tile_pool(name="zps", bufs=2, space="PSUM") as zps_p,
        ):
            wrec_t = const_p.tile([128, 2, 2, 2, 128], f16)
            nc.sync.dma_start(out=wrec_t[:], in_=wrec_d[:])
            id_t = const_p.tile([128, 128], f16)
            nc.sync.dma_start(out=id_t[:], in_=id_d[:])
            wout_t = const_p.tile([128, 2, 3], f16)
            nc.sync.dma_start(out=wout_t[:], in_=wout_d[:])
            l2acc = acc_p.tile([128, T // 2], f32)

            rv = None       # r_t (state entering current step), [128,2,128] AP
            rv_prev = None  # r_{t-1}
            sv = None       # s_{t-1}, tuple of half APs
            for ch in range(nch):
                t0 = ch * TCHUNK
                drive_t = drive_p.tile([128, TCHUNK, 2, 128], f16)
                nc.sync.dma_start(out=drive_t[:], in_=drive_d[:, t0 : t0 + TCHUNK, :, :])
                rh = r_p.tile([128, TCHUNK, 2, 128], f16)
                z_sb = z_p.tile([3, TCHUNK, 128], f32)
                ups_pair = {}
                for lt in range(TCHUNK):
                    t = t0 + lt
                    s_t = s_p.tile([128, 2, 128], f16, tag="s")
                    # one PSUM bank per step holds both hidden halves;
                    # pairs of banks are drive-injected together so the
                    # identity LDWEIGHTS is shared between two injects
                    if lt % 2 == 0:
                        for j in (0, 1):
                            upj = ups_p.tile([128, 2, 128], f32, tag="ups")
                            ups_pair[lt + j] = upj
                            nc.tensor.matmul(
                                upj[:], id_t[:], drive_t[:, lt + j, :, :],
                                start=True, stop=(sv is None and j == 0 and lt == 0),
                            )
                    up = ups_pair.pop(lt)
                    # r-matmuls: operand ready ~2 steps early
                    if rv_prev is not None:
                        for h in range(2):
                            for kc in range(2):
                                nc.tensor.matmul(
                                    up[:, h, :], wrec_t[:, 1, kc, h, :],
                                    rv_prev[:, kc, :],
                                    start=False, stop=False,
                                )
                    # s-matmuls: the chain edge (need s_{t-1})
                    if sv is not None:
                        for h in range(2):
                            for kc in range(2):
                                nc.tensor.matmul(
                                    up[:, h, :], wrec_t[:, 0, kc, h, :],
                                    sv[kc],
                                    start=False, stop=(h == 1 and kc == 1),
                                )
                    nc.scalar.activation(s_t[:], up[:], Relu)
                    # state update (off critical path): rh[t] = r_{t+1}
                    if rv is None:
                        nc.vector.tensor_copy(rh[:, lt, :, :], s_t[:])  # r_1 = s_0
                    else:
                        nc.vector.scalar_tensor_tensor(
                            out=rh[:, lt, :, :], in0=rv, scalar=BETA,
                            in1=s_t[:], op0=mult, op1=add,
                        )
                    rv_prev = rv  # None at t<=1 skips the r-MMs (r_0 = 0)
                    rv = rh[:, lt, :, :]
                    sv = (s_t[:, 0, :], s_t[:, 1, :])
                    # l2 partial sums per 2 steps (small pieces: avoid
                    # blocking the chain in either FIFO); alternate between
                    # VectorE (square via STT) and ScalarE (Square func),
                    # both with accum_out, to balance engine load
                    if lt % 2 == 1:
                        sq_t = sq_p.tile([128, 2, 2, 128], f16, tag="sq")
                        sl = rh[:, lt - 1 : lt + 1, :, :]
                        nc.vector.scalar_tensor_tensor(
                            out=sq_t[:], in0=sl, scalar=1.0, in1=sl,
                            op0=mult, op1=mult,
                            accum_out=l2acc[:, t // 2 : t // 2 + 1],
                        )
                    # z per completed 4-step window
                    if lt % 4 == 3:
                        q = lt // 4
                        zp = zps_p.tile([3, 4, 128], f32, tag="zp")
                        for kc in range(2):
                            nc.tensor.matmul(
                                zp[:],
                                wout_t[:, kc, :],
                                rh[:, 4 * q : 4 * q + 4, kc, :],
                                start=(kc == 0),
                                stop=(kc == 1),
                            )
                        nc.scalar.copy(z_sb[:, 4 * q : 4 * q + 4, :], zp[:])
                nc.sync.dma_start(out=z_d[:, t0 : t0 + TCHUNK, :], in_=z_sb[:])
            nc.sync.dma_start(out=l2_d[:], in_=l2acc[:])
    nc.compile()
    return nc


def _get_nc(T):
    if T not in _NC_CACHE:
        _NC_CACHE[T] = _build(T)
    return _NC_CACHE[T]


def _prep_maps(x, noise_seq, W_in, W_rec, W_out_w, bias):
    T = x.shape[1]
    a = np.float32(ALPHA)
    wrec = np.empty([128, 2, 2, 2, 128], np.float16)
    for kc in range(2):
        for h in range(2):
            blk = W_rec[h * 128 : (h + 1) * 128, kc * 128 : (kc + 1) * 128]
            wrec[:, 0, kc, h, :] = (a * blk.T).astype(np.float16)
            wrec[:, 1, kc, h, :] = (np.float32(BETA) * a * blk.T).astype(np.float16)
    ident = np.eye(128, dtype=np.float16)
    wout = np.empty([128, 2, 3], np.float16)
    wout[:, 0, :] = W_out_w[:, 0:128].T.astype(np.float16)
    wout[:, 1, :] = W_out_w[:, 128:256].T.astype(np.float16)

    in_maps = []
    for c in range(NCORES):
        bs = slice(c * BC, (c + 1) * BC)
        xw = np.matmul(x[bs], W_in.T.astype(np.float32))  # [128, T, 256]
        dr = a * (noise_seq[bs] + bias[None, None, :] + xw)
        drt = dr.transpose(2, 1, 0).reshape(2, 128, T, 128)  # [h, p, t, b]
        drive = np.ascontiguousarray(drt.transpose(1, 2, 0, 3), dtype=np.float16)
        in_maps.append(
            {"drive": drive, "wrec": wrec, "ident": ident, "wout": wout}
        )
    return in_maps


def install_profile_hook():
    """Register the axon NTFF profiling hook (the agent image lacks the
    antenv.axon_hooks module concourse expects; the C ABI is present in
    the injected libaxon_pjrt.so)."""
    import types

    if "antenv.axon_hooks" in sys.modules:
        return
    sys.path.insert(0, "/root/.axon_site")
    from trn_agent_boot import trn_boot

    hook = trn_boot._ntff_profile_via_ctypes("/opt/axon/libaxon_pjrt.so")
    mod = types.ModuleType("antenv.axon_hooks")
    mod.get_axon_ntff_profile_hook = lambda: hook
    mod.set_axon_ntff_profile_hook = lambda h: None
    sys.modules["antenv.axon_hooks"] = mod


def run(inputs, trace=False):
    """Returns ((out, l2_rates), BassKernelResults)."""
    from concourse.bass_utils import run_bass_kernel_spmd

    if trace:
        install_profile_hook()

    x = np.asarray(inputs["x"], np.float32)
    noise_seq = np.asarray(inputs["noise_seq"], np.float32)
    W_in = np.asarray(inputs["W_in"], np.float32)
    W_rec = np.asarray(inputs["W_rec"], np.float32)
    W_out_w = np.asarray(inputs["W_out_w"], np.float32)
    W_out_b = np.asarray(inputs["W_out_b"], np.float32)
    bias = np.asarray(inputs["bias"], np.float32)
    T = x.shape[1]
    Bt = x.shape[0]
    assert Bt == B_FULL, Bt

    nc = _get_nc(T)
    in_maps = _prep_maps(x, noise_seq, W_in, W_rec, W_out_w, bias)
    res = run_bass_kernel_spmd(
        nc, in_maps, core_ids=list(range(NCORES)), trace=trace
    )

    out = np.empty([B_FULL, T, N_OUT], np.float32)
    tot = 0.0
    for c in range(NCORES):
        z = res.results[c]["z"]  # [3, T, 128]
        out[c * BC : (c + 1) * BC] = z.transpose(2, 1, 0)
        tot += res.results[c]["l2"].sum(dtype=np.float64)
    out += W_out_b[None, None, :]
    l2 = np.float32(tot / (T * B_FULL * N_REC))
    return (out, l2), res


def kernel(**inputs):
    (out, l2), _ = run(inputs, trace=False)
    return out, l2
